# revision 1
# baseline (speedup 1.0000x reference)
"""Trainium2 Bass kernel for nn_Chan_spaAtt (SE-gated conv block).

Key observation: the spatial self-attention branch in the reference is dead
code -- `gamma*attn_out + xo` is discarded and the output depends only on
xo = x * sigmoid(xl + xg) through the final 3x3 conv + BN + ReLU.

Computation per sample (C=64, H=W=64, N=4096), BN affines folded host-side:
  t1   = relu(W1 @ x + b1)            [16, N]
  sarg = W2 @ t1 + (b2 + xg)          [64, N]
  xg   = G2 @ relu(G1 @ mean(x) + bg1) + bg2   [64, 1]
  xo   = x * sigmoid(sarg)            [64, N]
  y    = relu(conv3x3(xo, CW) + cb)   [64, N]

Sharding: pure data parallelism, one sample per NeuronCore (B=8, 8 cores).
On-chip layout: channels on partitions (64), spatial flat on free dim.
conv3x3 = 9 shifted matmuls over a zero-padded [64, 66*66-ish] xo buffer.
"""

import sys

if "/opt/trn_rl_repo" not in sys.path:
    sys.path.insert(0, "/opt/trn_rl_repo")

import numpy as np

import concourse.bass as bass
import concourse.bacc as bacc
import concourse.mybir as mybir
import concourse.tile as tile
from concourse.bass_utils import run_bass_kernel_spmd

B, C, H, W = 8, 64, 64, 64
N = H * W
INTER = 16
EPS = 1e-5
PW = W + 2          # padded row stride = 66
HEAD = PW + 1       # zeros before pixel (0,0) = 67
PAD_LEN = HEAD + PW * (H - 1) + W + HEAD  # = 67 + 63*66 + 64 + 67 = 4356
PAD_ALLOC = PAD_LEN + 2  # +2 slack so slice-then-rearrange stays in bounds
CHUNK = 512
NCHUNK = N // CHUNK  # 8
ROWS_PER_CHUNK = CHUNK // W  # 8

F32 = mybir.dt.float32
F32R = mybir.dt.float32r

# weights-blob column layout (f32r, 64 partitions)
O_W1T = 0
O_W2T = 16          # partitions 0:16
O_GW1T = 80
O_GW2T = 96         # partitions 0:16
O_CWT = 160
O_B1 = 736          # partitions 0:16
O_GB1 = 737         # partitions 0:16
O_BSIG = 738
O_CB = 739
WCOLS = 740
XPART = 1024        # x arrives in 4 quarters; quarter 0 rides in the blob DMA

_prog_cache = {}


def _pix(r, w):
    """Flat index of valid pixel (r, w) in the padded xo buffer."""
    return HEAD + r * PW + w


def build_program(n_cores=8):
    nc = bacc.Bacc("TRN2", debug=False, target_bir_lowering=False,
                   num_devices=n_cores)

    blob_d = nc.dram_tensor("blob", [C, WCOLS + XPART], F32R,
                            kind="ExternalInput").ap()
    xrest_d = nc.dram_tensor("xrest", [C, N - XPART], F32R,
                             kind="ExternalInput").ap()
    cwp_d = nc.dram_tensor("cwp", [2 * C, 3 * C], F32R,
                           kind="ExternalInput").ap()
    y_d = nc.dram_tensor("y", [C, N], F32, kind="ExternalOutput").ap()

    with tile.TileContext(nc) as tc:
        with tc.tile_pool(name="big", bufs=1) as bpool, \
             tc.tile_pool(name="work", bufs=3) as wpool, \
             tc.tile_pool(name="ps1p", bufs=2, space="PSUM") as pp1, \
             tc.tile_pool(name="ps2p", bufs=2, space="PSUM") as pp2, \
             tc.tile_pool(name="psyp", bufs=2, space="PSUM") as ppy:

            big = bpool.tile([C, WCOLS + N], F32R, tag="big")
            # DMA 1: weights + first x quarter -- a single semaphore gates
            # the first matmul (walrus allows only one sync wait per matmul).
            nc.sync.dma_start(big[:, 0:WCOLS + XPART], blob_d)
            for q in range(1, N // XPART):
                nc.sync.dma_start(
                    big[:, WCOLS + q * XPART: WCOLS + (q + 1) * XPART],
                    xrest_d[:, (q - 1) * XPART: q * XPART])

            w1t = big[:, O_W1T:O_W1T + INTER]
            w2t = big[0:INTER, O_W2T:O_W2T + C]
            gw1t = big[:, O_GW1T:O_GW1T + INTER]
            gw2t = big[0:INTER, O_GW2T:O_GW2T + C]
            cwt = big[:, O_CWT:O_CWT + 9 * C]
            b1 = big[0:INTER, O_B1:O_B1 + 1].bitcast(F32)
            gb1 = big[0:INTER, O_GB1:O_GB1 + 1].bitcast(F32)
            bsig = big[:, O_BSIG:O_BSIG + 1].bitcast(F32)
            cb = big[:, O_CB:O_CB + 1].bitcast(F32)
            x_sb = big[:, WCOLS:WCOLS + N]

            cwp = bpool.tile([2 * C, 3 * C], F32R, tag="cwp")
            nc.sync.dma_start(cwp[:], cwp_d)

            # ---- padded xo buffer; zero the halo regions ----
            # partitions 0:64 hold xo_pad (copy A); partitions 64:128 hold the
            # same data shifted left by 2*PW (copy B) so one K=128 matmul sums
            # the dy=-1 (A) and dy=+1 (B) conv taps at a single rhs offset.
            xo_pad = bpool.tile([2 * C, PAD_ALLOC], F32R, tag="xopad")
            nc.vector.memset(xo_pad[:].bitcast(mybir.dt.uint32), 0)

            # ---- mm1 + relu, chunk 0 first (PE head: observes blob DMA) ----
            mm1_insts, mm2_insts = [], []
            t1s = {}

            def emit_mm1(ci):
                xc = x_sb[:, ci * CHUNK:(ci + 1) * CHUNK]
                ps1 = pp1.tile([INTER, CHUNK], F32, tag="ps1")
                mm1_insts.append(nc.tensor.matmul(ps1[:], w1t, xc,
                                                  start=True, stop=True))
                t1 = wpool.tile([INTER, CHUNK], F32R, tag="t1")
                nc.scalar.activation(t1[:], ps1[:],
                                     mybir.ActivationFunctionType.Relu,
                                     bias=b1)
                t1s[ci] = t1

            emit_mm1(0)

            # ---- global branch: xg folded into per-channel sigmoid bias D ----
            g_parts = wpool.tile([C, 4], F32, tag="gparts")
            for q in range(4):
                nc.vector.reduce_sum(
                    g_parts[:, q:q + 1],
                    x_sb.bitcast(F32)[:, q * XPART:(q + 1) * XPART],
                    axis=mybir.AxisListType.X)
            g_raw = wpool.tile([C, 1], F32, tag="graw")
            nc.vector.reduce_sum(g_raw[:], g_parts[:],
                                 axis=mybir.AxisListType.X)
            ps_g1 = pp1.tile([INTER, 1], F32, tag="ps1")
            gmm1 = nc.tensor.matmul(ps_g1[:], gw1t.bitcast(F32), g_raw[:],
                                    start=True, stop=True)
            bass._add_dep_helper(gmm1.ins, mm1_insts[0].ins, sync=False,
                                 reason="PE observes blob DMA via mm1[0]")
            g1 = wpool.tile([INTER, 1], F32, tag="g1")
            nc.scalar.activation(g1[:], ps_g1[:],
                                 mybir.ActivationFunctionType.Relu,
                                 bias=gb1, scale=1.0 / N)
            ps_g2 = pp2.tile([C, 1], F32, tag="ps2")
            gmm2 = nc.tensor.matmul(ps_g2[:], gw2t.bitcast(F32), g1[:],
                                    start=True, stop=True)
            dbias = wpool.tile([C, 1], F32, tag="dbias")
            nc.scalar.activation(dbias[:], ps_g2[:],
                                 mybir.ActivationFunctionType.Identity,
                                 bias=bsig)

            # ---- phase 1: SE gating, chunked over spatial ----
            for ci in range(NCHUNK):
                if ci > 0:
                    emit_mm1(ci)
                xc = x_sb[:, ci * CHUNK:(ci + 1) * CHUNK]
                ps2 = pp2.tile([C, CHUNK], F32, tag="ps2")
                mm2_insts.append(nc.tensor.matmul(ps2[:], w2t, t1s.pop(ci)[:],
                                                  start=True, stop=True))
                if ci >= 2:
                    bass._add_dep_helper(
                        mm1_insts[ci].ins, mm2_insts[ci - 2].ins, sync=False,
                        reason="pipeline order: ps1 slot release observed")
                sig = wpool.tile([C, CHUNK], F32, tag="sig")
                nc.scalar.activation(sig[:], ps2[:],
                                     mybir.ActivationFunctionType.Sigmoid,
                                     bias=dbias[:])
                r0 = ci * ROWS_PER_CHUNK
                dst = xo_pad[0:C, _pix(r0, 0): _pix(r0, 0) + ROWS_PER_CHUNK * PW]
                dst = dst.rearrange("p (r w) -> p r w", w=PW)[:, :, 0:W]
                xcr = xc.bitcast(F32).rearrange("p (r w) -> p r w", w=W)
                sigr = sig[:].rearrange("p (r w) -> p r w", w=W)
                nc.vector.tensor_mul(dst, xcr, sigr)
                s0 = max(2 * PW, _pix(r0, 0))
                e0 = _pix(r0 + ROWS_PER_CHUNK - 1, W)
                nc.sync.dma_start(xo_pad[C:2 * C, s0 - 2 * PW:e0 - 2 * PW],
                                  xo_pad[0:C, s0:e0])

            # ---- phase 2: 3x3 conv as 3 paired + 3 single matmuls ----
            def shifted_rhs(parts, o):
                rhs = xo_pad[0:parts, o: o + ROWS_PER_CHUNK * PW]
                return rhs.rearrange("p (r w) -> p r w", w=PW)[:, :, 0:W]

            for cj in range(NCHUNK // 2):
                psy = ppy.tile([C, 2 * CHUNK], F32, tag="psy")
                for h in range(2):
                    r0 = (2 * cj + h) * ROWS_PER_CHUNK
                    half = psy[:, h * CHUNK:(h + 1) * CHUNK]
                    for j, dx in enumerate((-1, 0, 1)):
                        nc.tensor.matmul(half, cwp[:, j * C:(j + 1) * C],
                                         shifted_rhs(2 * C, _pix(r0 - 1, dx)),
                                         start=(j == 0), stop=False)
                    for j, dx in enumerate((-1, 0, 1)):
                        k = 3 + (dx + 1)
                        nc.tensor.matmul(half, cwt[:, k * C:(k + 1) * C],
                                         shifted_rhs(C, _pix(r0, dx)),
                                         start=False, stop=(j == 2))
                ybuf = wpool.tile([C, 2 * CHUNK], F32, tag="ybuf")
                nc.scalar.activation(ybuf[:], psy[:],
                                     mybir.ActivationFunctionType.Relu,
                                     bias=cb)
                nc.sync.dma_start(
                    y_d[:, 2 * cj * CHUNK:(2 * cj + 2) * CHUNK], ybuf[:])

    nc.compile()
    return nc


def _affine(s, b, m, v):
    inv = s / np.sqrt(v + EPS)
    return inv, b - m * inv


def prepare_weights(inputs):
    f = lambda k: np.asarray(inputs[k], dtype=np.float32)
    a1, c1 = _affine(f("ls1"), f("lbb1"), f("lm1"), f("lv1"))
    W1 = a1[:, None] * f("lw1")
    B1 = a1 * f("lb1") + c1
    a2, c2 = _affine(f("ls2"), f("lbb2"), f("lm2"), f("lv2"))
    W2 = a2[:, None] * f("lw2")
    B2 = a2 * f("lb2") + c2
    ag1, cg1 = _affine(f("gs1"), f("gbb1"), f("gm1"), f("gv1"))
    G1 = ag1[:, None] * f("gw1")
    Bg1 = ag1 * f("gb1") + cg1
    ag2, cg2 = _affine(f("gs2"), f("gbb2"), f("gm2"), f("gv2"))
    G2 = ag2[:, None] * f("gw2")
    Bg2 = ag2 * f("gb2") + cg2
    ac, cc = _affine(f("cs"), f("cbb"), f("cm"), f("cv"))
    CW = ac[:, None, None, None] * f("cw")        # [O, C, 3, 3]
    CB = ac * f("cb") + cc
    cwt = np.ascontiguousarray(
        CW.transpose(1, 2, 3, 0).reshape(C, 9 * C))  # [c, (ky kx) o]
    col = lambda v: np.ascontiguousarray(v.reshape(-1, 1), dtype=np.float32)
    cn = lambda v: np.ascontiguousarray(v, dtype=np.float32)
    return {
        "w1t": cn(W1.T), "b1": col(B1),
        "w2t": cn(W2.T),
        "gw1t": cn(G1.T), "gb1": col(Bg1),
        "gw2t": cn(G2.T), "bsig": col(B2 + Bg2),
        "cwt": cn(cwt), "cb": col(CB),
    }


def assemble_wblob(shared):
    wb = np.zeros((C, WCOLS), np.float32)
    wb[:, O_W1T:O_W1T + INTER] = shared["w1t"]
    wb[0:INTER, O_W2T:O_W2T + C] = shared["w2t"]
    wb[:, O_GW1T:O_GW1T + INTER] = shared["gw1t"]
    wb[0:INTER, O_GW2T:O_GW2T + C] = shared["gw2t"]
    wb[:, O_CWT:O_CWT + 9 * C] = shared["cwt"]
    wb[0:INTER, O_B1] = shared["b1"][:, 0]
    wb[0:INTER, O_GB1] = shared["gb1"][:, 0]
    wb[:, O_BSIG] = shared["bsig"][:, 0]
    wb[:, O_CB] = shared["cb"][:, 0]
    return wb


def assemble_cwp(shared):
    # cwt[c, (ky*3+kx)*64 + o]; pairs stack ky=0 on top, ky=2 below, per kx
    cwt = shared["cwt"]
    cwp = np.zeros((2 * C, 3 * C), np.float32)
    for j in range(3):
        cwp[0:C, j * C:(j + 1) * C] = cwt[:, (0 + j) * C:(0 + j + 1) * C]
        cwp[C:2 * C, j * C:(j + 1) * C] = cwt[:, (6 + j) * C:(6 + j + 1) * C]
    return cwp


def make_core_inputs(inputs):
    shared = prepare_weights(inputs)
    wb = assemble_wblob(shared)
    cwp = np.ascontiguousarray(assemble_cwp(shared))
    x = np.asarray(inputs["x"], dtype=np.float32)
    maps = []
    for i in range(B):
        xi = x[i].reshape(C, N)
        maps.append({
            "blob": np.ascontiguousarray(
                np.concatenate([wb, xi[:, :XPART]], axis=1)),
            "xrest": np.ascontiguousarray(xi[:, XPART:]),
            "cwp": cwp,
        })
    return maps


def _run(inputs, trace=False):
    in_maps = make_core_inputs(inputs)
    if "prog" not in _prog_cache:
        _prog_cache["prog"] = build_program(B)
    nc = _prog_cache["prog"]
    res = run_bass_kernel_spmd(nc, in_maps, list(range(B)), trace=trace)
    out = np.stack([r["y"].reshape(C, H, W) for r in res.results])
    return out.astype(np.float32), res


def kernel(**inputs):
    out, _ = _run(inputs, trace=False)
    return out


def kernel_traced(inputs):
    return _run(inputs, trace=True)


def reference_numpy(inputs):
    """Pure-numpy emulation of the (dead-code-eliminated) reference, using the
    same folded weights as the device kernel. For algebra validation only."""
    shared = prepare_weights(inputs)
    x = np.asarray(inputs["x"], dtype=np.float32)  # [B, C, H, W]
    f = lambda k: np.asarray(inputs[k], dtype=np.float32)
    a1, c1 = _affine(f("ls1"), f("lbb1"), f("lm1"), f("lv1"))
    B1 = a1 * f("lb1") + c1
    out = np.empty_like(x)
    for i in range(B):
        xs = x[i].reshape(C, N)
        t1 = np.maximum(shared["w1t"].T @ xs + B1[:, None], 0.0)
        g = xs.mean(axis=1, keepdims=True)
        g1 = np.maximum(shared["gw1t"].T @ g + shared["gb1"], 0.0)
        d = shared["gw2t"].T @ g1 + shared["bsig"]
        sarg = shared["w2t"].T @ t1 + d
        xo = xs * (1.0 / (1.0 + np.exp(-sarg)))
        xop = np.zeros((C, H + 2, W + 2), np.float32)
        xop[:, 1:-1, 1:-1] = xo.reshape(C, H, W)
        y = np.zeros((C, N), np.float32)
        for k in range(9):
            ky, kx = divmod(k, 3)
            sh = xop[:, ky:ky + H, kx:kx + W].reshape(C, N)
            y += shared["cwt"][:, k * C:(k + 1) * C].T @ sh
        y = np.maximum(y + shared["cb"], 0.0)
        out[i] = y.reshape(C, H, W)
    return out



# revision 4
# speedup vs baseline: 1.2950x; 1.2950x over previous
"""Trainium2 Bass kernel for nn_Chan_spaAtt (SE-gated conv block).

The spatial self-attention branch in the reference is dead code -- the
output depends only on xo = x * sigmoid(xl + xg) through the final
3x3 conv + BN + ReLU.  BN affines are folded host-side.

Per sample (C=64, H=W=64), with an even/odd COLUMN-PARITY layout:
  partitions 0:64  = channel c of even image columns (pair index j -> col 2j)
  partitions 64:128 = channel c of odd image columns  (j -> col 2j+1)

  mm1:  t1   = relu(blockdiag(W1,W1) @ x + b1)       [32, N/2]
  mm2:  sarg = blockstack(W2,W2) @ t1                [128, N/2]
  xg (global branch) folded into the sigmoid bias via two tiny matmuls
  xo   = x * sigmoid(sarg + dbias)                   [128, N/2]

3x3 conv as 6 matmuls of N/2 rows (vs 9 at N): per row-tap dy the
dense matmul A_dy covers 4 tap-instances (dx=0,+1 for even outputs,
dx=-1,0 for odd) reading buf1 = xo; matmul B_dy covers the remaining 2
(dx=-1 even, dx=+1 odd) reading buf2 = half-swapped/column-shifted copy
of buf1 built by two contiguous SBUF->SBUF DMAs per chunk (row pads in
buf1 propagate the zero boundary columns automatically).

Rows live on the free axis with one zero pad row above/below and 2 pad
slots per 32-pair row (stride 34), so all dy/dx shifts are plain AP
offsets.  Everything computes in bf16 (inputs/outputs cast host-side),
PSUM accumulation in fp32; rel err ~4e-3 vs fp32 reference.

Sharding: pure data parallelism, one sample per NeuronCore (B=8).
"""

import sys

if "/opt/trn_rl_repo" not in sys.path:
    sys.path.insert(0, "/opt/trn_rl_repo")

import numpy as np
import ml_dtypes

import concourse.bass as bass
import concourse.bacc as bacc
import concourse.mybir as mybir
import concourse.tile as tile
from concourse.bass_utils import run_bass_kernel_spmd

B, C, H, W = 8, 64, 64, 64
N = H * W
NP = N // 2          # pixels per parity = 2048
INTER = 16
EPS = 1e-5
JP = W // 2          # pairs per row = 32
RSTR = JP + 2        # buf row stride = 34 (pad_l, 32 slots, pad_r)
NROW = H + 2         # 66 buffered rows (zero row above/below)
BUFCOLS = NROW * RSTR + 4   # 2248 incl. slack
CHUNK = 512
NCHUNK = NP // CHUNK  # 4
ROWS_PER_CHUNK = CHUNK // JP  # 16

BF16 = mybir.dt.bfloat16
F32 = mybir.dt.float32

# blob (bf16) column layout
O_W1B = 0            # [128, 32]
O_W2B = 32           # [32, 128]
O_CONV = 160         # 6 x [128, 128]: A(-1), A(0), A(+1), B(-1), B(0), B(+1)
BLOBCOLS = 160 + 6 * 128     # 928
BLOB_A_COLS = 160    # first DMA: mm1/mm2 weights

# aux (f32) column layout
O_B1 = 0             # rows 0:32
O_GB1 = 1            # rows 0:16
O_BSIG = 2           # rows 0:128
O_CB = 3             # rows 0:128
O_GW1B = 4           # [128, 16]
O_GW2B = 20          # [16, 128]
AUXCOLS = 20 + 128   # 148

N_FILLERS = 4        # PE p-state warmers (scratch matmuls on x data)

_prog_cache = {}


def _row(r):
    """Flat offset of image row r's pad_l in buf1/buf2 (rows -1..64)."""
    return (r + 1) * RSTR


def build_program(n_cores=8):
    nc = bacc.Bacc("TRN2", debug=False, target_bir_lowering=False,
                   num_devices=n_cores)

    xin_d = nc.dram_tensor("xin", [2 * C, NP], BF16, kind="ExternalInput").ap()
    blob_d = nc.dram_tensor("blob", [2 * C, BLOBCOLS], BF16,
                            kind="ExternalInput").ap()
    aux_d = nc.dram_tensor("aux", [2 * C, AUXCOLS], F32,
                           kind="ExternalInput").ap()
    y_d = nc.dram_tensor("y", [2 * C, NP], BF16, kind="ExternalOutput").ap()

    with tile.TileContext(nc) as tc:
        with tc.tile_pool(name="big", bufs=1) as bpool, \
             tc.tile_pool(name="t1p", bufs=4) as t1pool, \
             tc.tile_pool(name="sigp", bufs=2) as sigpool, \
             tc.tile_pool(name="ybp", bufs=2) as ybpool, \
             tc.tile_pool(name="ps1p", bufs=2, space="PSUM") as pp1, \
             tc.tile_pool(name="ps2p", bufs=2, space="PSUM") as pp2, \
             tc.tile_pool(name="psyp", bufs=4, space="PSUM") as ppy:

            xsb = bpool.tile([2 * C, NP], BF16, tag="xsb")
            wsb = bpool.tile([2 * C, BLOBCOLS], BF16, tag="wsb")
            auxsb = bpool.tile([2 * C, AUXCOLS], F32, tag="auxsb")
            buf1 = bpool.tile([2 * C, BUFCOLS], BF16, tag="buf1")
            buf2 = bpool.tile([2 * C, BUFCOLS], BF16, tag="buf2")
            gparts = bpool.tile([2 * C, NCHUNK], F32, tag="gparts")
            gsum = bpool.tile([2 * C, 1], F32, tag="gsum")
            g1 = bpool.tile([INTER, 1], F32, tag="g1")
            dbias = bpool.tile([2 * C, 1], F32, tag="dbias")

            # ---- DMAs: x chunks first (global mean gates everything),
            # then mm weights, aux, conv weights ----
            for ci in range(NCHUNK):
                nc.sync.dma_start(xsb[:, ci * CHUNK:(ci + 1) * CHUNK],
                                  xin_d[:, ci * CHUNK:(ci + 1) * CHUNK])
            nc.sync.dma_start(wsb[:, 0:BLOB_A_COLS], blob_d[:, 0:BLOB_A_COLS])
            nc.sync.dma_start(auxsb[:], aux_d)
            nc.sync.dma_start(wsb[:, BLOB_A_COLS:BLOBCOLS],
                              blob_d[:, BLOB_A_COLS:BLOBCOLS])

            w1b = wsb[:, O_W1B:O_W1B + 32]
            w2b = wsb[0:32, O_W2B:O_W2B + 128]
            convw = [wsb[:, O_CONV + k * 128:O_CONV + (k + 1) * 128]
                     for k in range(6)]   # A-1 A0 A+1 B-1 B0 B+1
            b1 = auxsb[0:32, O_B1:O_B1 + 1]
            gb1 = auxsb[0:INTER, O_GB1:O_GB1 + 1]
            bsig = auxsb[:, O_BSIG:O_BSIG + 1]
            cb = auxsb[:, O_CB:O_CB + 1]
            gw1b = auxsb[:, O_GW1B:O_GW1B + INTER]
            gw2b = auxsb[0:INTER, O_GW2B:O_GW2B + 128]

            # ---- zero pads (DVE): rows -1/64 in both bufs + buf1 slot pads
            # (buf1's pads propagate zeros into buf2 via the swap copies) ----
            nc.vector.memset(buf1[:, 0:RSTR].bitcast(mybir.dt.uint16), 0)
            nc.vector.memset(
                buf1[:, _row(H):_row(H) + RSTR].bitcast(mybir.dt.uint16), 0)
            inner = buf1[:, RSTR:RSTR + H * RSTR].rearrange(
                "p (r w) -> p r w", w=RSTR)
            nc.vector.memset(inner[:, :, 0:1].bitcast(mybir.dt.uint16), 0)
            nc.vector.memset(
                inner[:, :, RSTR - 1:RSTR].bitcast(mybir.dt.uint16), 0)
            nc.vector.memset(buf2[:, 0:RSTR].bitcast(mybir.dt.uint16), 0)
            nc.vector.memset(
                buf2[:, _row(H):_row(H) + RSTR].bitcast(mybir.dt.uint16), 0)

            # ---- global mean partial reduces (DVE), as x chunks land ----
            for ci in range(NCHUNK):
                nc.vector.reduce_sum(gparts[:, ci:ci + 1],
                                     xsb[:, ci * CHUNK:(ci + 1) * CHUNK],
                                     axis=mybir.AxisListType.X)
            nc.vector.reduce_sum(gsum[:], gparts[:], axis=mybir.AxisListType.X)

            # ---- mm1 all chunks (PE front-fill); t1relu 0/1 on Act ----
            ps1s, t1s = {}, {}
            for ci in range(NCHUNK):
                ps1 = pp1.tile([32, CHUNK], F32, tag="ps1")
                nc.tensor.matmul(ps1[:], w1b,
                                 xsb[:, ci * CHUNK:(ci + 1) * CHUNK],
                                 start=True, stop=True)
                ps1s[ci] = ps1
                t1 = t1pool.tile([32, CHUNK], BF16, tag="t1")
                t1s[ci] = t1
                if ci < 2:   # Act, early idle window
                    nc.scalar.activation(t1[:], ps1[:],
                                         mybir.ActivationFunctionType.Relu,
                                         bias=b1)

            # ---- global branch: dbias = G2 @ relu(G1 @ mean + gb1) + bsig ----
            ps_g1 = pp1.tile([INTER, 1], F32, tag="ps1")
            nc.tensor.matmul(ps_g1[:], gw1b, gsum[:], start=True, stop=True)
            nc.scalar.activation(g1[:], ps_g1[:],
                                 mybir.ActivationFunctionType.Relu,
                                 bias=gb1, scale=1.0 / N)
            ps_g2 = pp2.tile([2 * C, 1], F32, tag="ps2")
            nc.tensor.matmul(ps_g2[:], gw2b, g1[:], start=True, stop=True)
            nc.scalar.activation(dbias[:], ps_g2[:],
                                 mybir.ActivationFunctionType.Identity,
                                 bias=bsig)

            # ---- t1relu 2/3 on DVE (keeps Act free for sigmoid chain) ----
            for ci in (2, 3):
                nc.vector.tensor_scalar(t1s[ci][:], ps1s[ci][:], b1, 0.0,
                                        mybir.AluOpType.add,
                                        mybir.AluOpType.max)

            # ---- mm2 + sigmoid + gated mul into buf1, swap into buf2 ----
            def fill():
                psf = ppy.tile([2 * C, CHUNK], F32, tag="psy")
                nc.tensor.matmul(psf[:], xsb[:, 0:128], xsb[:, 0:CHUNK],
                                 start=True, stop=True)

            for ci in range(NCHUNK):
                ps2 = pp2.tile([2 * C, CHUNK], F32, tag="ps2")
                nc.tensor.matmul(ps2[:], w2b, t1s[ci][:],
                                 start=True, stop=True)
                if N_FILLERS > ci:
                    fill()
                sig = sigpool.tile([2 * C, CHUNK], BF16, tag="sig")
                nc.scalar.activation(sig[:], ps2[:],
                                     mybir.ActivationFunctionType.Sigmoid,
                                     bias=dbias[:])
                r0 = ci * ROWS_PER_CHUNK
                dst = buf1[:, _row(r0):_row(r0) + ROWS_PER_CHUNK * RSTR]
                dst = dst.rearrange("p (r w) -> p r w", w=RSTR)[:, :, 1:JP + 1]
                xcr = xsb[:, ci * CHUNK:(ci + 1) * CHUNK].rearrange(
                    "p (r w) -> p r w", w=JP)
                sgr = sig[:].rearrange("p (r w) -> p r w", w=JP)
                nc.vector.tensor_mul(dst, xcr, sgr)
                # swap halves into buf2 (flat contiguous, pads carry zeros):
                #   buf2_low slot j = xo_odd[j-1]; buf2_high slot j = xo_even[j+1]
                s0 = _row(r0) + 1
                ln = ROWS_PER_CHUNK * RSTR
                # chunk 0 starts one element earlier so buf2_low row0/slot0
                # picks up buf1's zero pad
                ext = 1 if ci == 0 else 0
                nc.sync.dma_start(buf2[0:C, s0 + 1 - ext:s0 + 1 + ln],
                                  buf1[C:2 * C, s0 - ext:s0 + ln])
                nc.sync.dma_start(buf2[C:2 * C, s0 - 1:s0 - 1 + ln],
                                  buf1[0:C, s0:s0 + ln])

            # ---- conv: 3 dense A matmuls + 3 half B matmuls per chunk ----
            def rhs(buf, r0, dy):
                v = buf[:, _row(r0 + dy) + 1:
                        _row(r0 + dy) + 1 + ROWS_PER_CHUNK * RSTR]
                return v.rearrange("p (r w) -> p r w", w=RSTR)[:, :, 0:JP]

            psys = {}
            for ci in range(NCHUNK):
                r0 = ci * ROWS_PER_CHUNK
                psy = ppy.tile([2 * C, CHUNK], F32, tag="psy")
                psys[ci] = psy
                for j, dy in enumerate((-1, 0, 1)):
                    nc.tensor.matmul(psy[:], convw[j], rhs(buf1, r0, dy),
                                     start=(j == 0), stop=False)
            for ci in range(NCHUNK):
                r0 = ci * ROWS_PER_CHUNK
                psy = psys[ci]
                for j, dy in enumerate((-1, 0, 1)):
                    nc.tensor.matmul(psy[:], convw[3 + j], rhs(buf2, r0, dy),
                                     start=False, stop=(j == 2))
                ybuf = ybpool.tile([2 * C, CHUNK], BF16, tag="ybuf")
                nc.scalar.activation(ybuf[:], psy[:],
                                     mybir.ActivationFunctionType.Relu,
                                     bias=cb)
                nc.sync.dma_start(y_d[:, ci * CHUNK:(ci + 1) * CHUNK],
                                  ybuf[:])

    nc.compile()
    return nc


def _affine(s, b, m, v):
    inv = s / np.sqrt(v + EPS)
    return inv, b - m * inv


def prepare_weights(inputs):
    f = lambda k: np.asarray(inputs[k], dtype=np.float32)
    a1, c1 = _affine(f("ls1"), f("lbb1"), f("lm1"), f("lv1"))
    W1 = a1[:, None] * f("lw1")                    # [16, 64]
    B1 = a1 * f("lb1") + c1
    a2, c2 = _affine(f("ls2"), f("lbb2"), f("lm2"), f("lv2"))
    W2 = a2[:, None] * f("lw2")                    # [64, 16]
    B2 = a2 * f("lb2") + c2
    ag1, cg1 = _affine(f("gs1"), f("gbb1"), f("gm1"), f("gv1"))
    G1 = ag1[:, None] * f("gw1")                   # [16, 64]
    Bg1 = ag1 * f("gb1") + cg1
    ag2, cg2 = _affine(f("gs2"), f("gbb2"), f("gm2"), f("gv2"))
    G2 = ag2[:, None] * f("gw2")                   # [64, 16]
    Bg2 = ag2 * f("gb2") + cg2
    ac, cc = _affine(f("cs"), f("cbb"), f("cm"), f("cv"))
    CW = ac[:, None, None, None] * f("cw")         # [O, C, 3, 3]
    CB = ac * f("cb") + cc
    return dict(W1=W1, B1=B1, W2=W2, G1=G1, Bg1=Bg1, G2=G2,
                bsig=B2 + Bg2, CW=CW, CB=CB)


def assemble_blob(sh):
    blob = np.zeros((2 * C, BLOBCOLS), np.float32)
    W1T = sh["W1"].T                               # [64, 16]
    blob[0:C, O_W1B:O_W1B + INTER] = W1T
    blob[C:2 * C, O_W1B + INTER:O_W1B + 32] = W1T
    W2T = sh["W2"].T                               # [16, 64]
    blob[0:INTER, O_W2B:O_W2B + C] = W2T
    blob[INTER:32, O_W2B + C:O_W2B + 2 * C] = W2T
    CW = sh["CW"]
    cwt = lambda dy, dx: CW[:, :, dy + 1, dx + 1].T   # [c, o]
    for j, dy in enumerate((-1, 0, 1)):
        A = np.zeros((2 * C, 2 * C), np.float32)
        A[0:C, 0:C] = cwt(dy, 0)
        A[C:2 * C, 0:C] = cwt(dy, 1)
        A[0:C, C:2 * C] = cwt(dy, -1)
        A[C:2 * C, C:2 * C] = cwt(dy, 0)
        blob[:, O_CONV + j * 128:O_CONV + (j + 1) * 128] = A
        Bm = np.zeros((2 * C, 2 * C), np.float32)
        Bm[0:C, 0:C] = cwt(dy, -1)
        Bm[C:2 * C, C:2 * C] = cwt(dy, 1)
        blob[:, O_CONV + (3 + j) * 128:O_CONV + (4 + j) * 128] = Bm
    return blob.astype(ml_dtypes.bfloat16)


def assemble_aux(sh):
    aux = np.zeros((2 * C, AUXCOLS), np.float32)
    aux[0:INTER, O_B1] = sh["B1"]
    aux[INTER:32, O_B1] = sh["B1"]
    aux[0:INTER, O_GB1] = sh["Bg1"]
    aux[0:C, O_BSIG] = sh["bsig"]
    aux[C:2 * C, O_BSIG] = sh["bsig"]
    aux[0:C, O_CB] = sh["CB"]
    aux[C:2 * C, O_CB] = sh["CB"]
    G1T = sh["G1"].T                               # [64, 16]
    aux[0:C, O_GW1B:O_GW1B + INTER] = G1T
    aux[C:2 * C, O_GW1B:O_GW1B + INTER] = G1T
    G2T = sh["G2"].T                               # [16, 64]
    aux[0:INTER, O_GW2B:O_GW2B + C] = G2T
    aux[0:INTER, O_GW2B + C:O_GW2B + 2 * C] = G2T
    return np.ascontiguousarray(aux)


def pack_x(xi):
    """[C, H, W] f32 -> [128, NP] bf16 parity-split."""
    ev = xi[:, :, 0::2].reshape(C, NP)
    od = xi[:, :, 1::2].reshape(C, NP)
    return np.ascontiguousarray(
        np.concatenate([ev, od], axis=0)).astype(ml_dtypes.bfloat16)


def unpack_y(yc):
    """[128, NP] bf16 -> [C, H, W] f32."""
    y = np.empty((C, H, W), np.float32)
    y[:, :, 0::2] = np.asarray(yc[0:C], np.float32).reshape(C, H, JP)
    y[:, :, 1::2] = np.asarray(yc[C:2 * C], np.float32).reshape(C, H, JP)
    return y


def make_core_inputs(inputs):
    sh = prepare_weights(inputs)
    blob = assemble_blob(sh)
    aux = assemble_aux(sh)
    x = np.asarray(inputs["x"], dtype=np.float32)
    return [{"xin": pack_x(x[i]), "blob": blob, "aux": aux}
            for i in range(B)]


def _run(inputs, trace=False):
    in_maps = make_core_inputs(inputs)
    if "prog" not in _prog_cache:
        _prog_cache["prog"] = build_program(B)
    nc = _prog_cache["prog"]
    res = run_bass_kernel_spmd(nc, in_maps, list(range(B)), trace=trace)
    out = np.stack([unpack_y(r["y"]) for r in res.results])
    return out.astype(np.float32), res


def kernel(**inputs):
    out, _ = _run(inputs, trace=False)
    return out


def kernel_traced(inputs):
    return _run(inputs, trace=True)


def reference_numpy(inputs):
    """Numpy emulation of the device algebra (parity layout, bf16 casts)."""
    bf = lambda a: a.astype(ml_dtypes.bfloat16).astype(np.float32)
    sh = prepare_weights(inputs)
    blob = np.asarray(assemble_blob(sh), np.float32)
    aux = assemble_aux(sh)
    x = np.asarray(inputs["x"], dtype=np.float32)
    out = np.empty_like(x)
    w1b = blob[:, O_W1B:O_W1B + 32]
    w2b = blob[0:32, O_W2B:O_W2B + 128]
    convw = [blob[:, O_CONV + k * 128:O_CONV + (k + 1) * 128]
             for k in range(6)]
    for i in range(B):
        xp = np.asarray(pack_x(x[i]), np.float32)      # [128, NP]
        gs = xp.sum(axis=1, keepdims=True)             # [128, 1]
        g1 = np.maximum(aux[:, O_GW1B:O_GW1B + INTER][0:128].T @ gs / N
                        + aux[0:INTER, O_GB1:O_GB1 + 1], 0.0)
        db = aux[0:INTER, O_GW2B:O_GW2B + 128].T @ g1 \
            + aux[:, O_BSIG:O_BSIG + 1]
        t1 = bf(np.maximum(w1b.T @ xp + aux[0:32, O_B1:O_B1 + 1], 0.0))
        sarg = w2b.T @ t1 + db
        sig = bf(1.0 / (1.0 + np.exp(-sarg)))
        xo = bf(xp * sig)
        # padded buffers
        b1_ = np.zeros((128, NROW * RSTR + 4), np.float32)
        v = b1_[:, RSTR:RSTR + H * RSTR].reshape(128, H, RSTR)
        v[:, :, 1:JP + 1] = xo.reshape(128, H, JP)
        b2_ = np.zeros_like(b1_)
        s0 = RSTR + 1
        ln = H * RSTR
        b2_[0:C, s0 + 1:s0 + 1 + ln] = b1_[C:2 * C, s0:s0 + ln]
        b2_[C:2 * C, s0 - 1:s0 - 1 + ln] = b1_[0:C, s0:s0 + ln]
        y = np.zeros((128, NP), np.float32)
        for j, dy in enumerate((-1, 0, 1)):
            for bb, wb in ((b1_, convw[j]), (b2_, convw[3 + j])):
                sh_v = bb[:, (1 + dy) * RSTR + 1:
                          (1 + dy) * RSTR + 1 + H * RSTR]
                sh_v = sh_v.reshape(128, H, RSTR)[:, :, 0:JP].reshape(128, NP)
                y += wb.T @ bf(sh_v)
        y = np.maximum(y + aux[:, O_CB:O_CB + 1], 0.0)
        out[i] = unpack_y(y.astype(ml_dtypes.bfloat16))
    return out


# revision 12
# speedup vs baseline: 1.5124x; 1.1679x over previous
"""Trainium2 Bass kernel for nn_Chan_spaAtt (SE-gated conv block).

The spatial self-attention branch in the reference is dead code -- the
output depends only on xo = x * sigmoid(xl + xg) through the final
3x3 conv + BN + ReLU.  BN affines are folded host-side.

Per sample (C=64, H=W=64), with an even/odd COLUMN-PARITY layout:
  partitions 0:64  = channel c of even image columns (pair index j -> col 2j)
  partitions 64:128 = channel c of odd image columns  (j -> col 2j+1)

  mm1:  t1   = relu(blockdiag(W1,W1) @ x + b1)       [32, N/2]
  mm2:  sarg = blockstack(W2,W2) @ t1                [128, N/2]
  xg (global branch) folded into the sigmoid bias via two tiny matmuls
  xo   = x * sigmoid(sarg + dbias)                   [128, N/2]

3x3 conv as 6 matmuls of N/2 rows (vs 9 at N): per row-tap dy the
dense matmul A_dy covers 4 tap-instances (dx=0,+1 for even outputs,
dx=-1,0 for odd) reading buf1 = xo; matmul B_dy covers the remaining 2
(dx=-1 even, dx=+1 odd) reading buf2 = half-swapped/column-shifted copy
of buf1 built by two contiguous SBUF->SBUF DMAs per chunk (row pads in
buf1 propagate the zero boundary columns automatically).

Rows live on the free axis with one zero pad row above/below and 2 pad
slots per 32-pair row (stride 34), so all dy/dx shifts are plain AP
offsets.  Everything computes in bf16 (inputs/outputs cast host-side),
PSUM accumulation in fp32; rel err ~4e-3 vs fp32 reference.

Sharding: pure data parallelism, one sample per NeuronCore (B=8).
"""

import sys

if "/opt/trn_rl_repo" not in sys.path:
    sys.path.insert(0, "/opt/trn_rl_repo")

import numpy as np
import ml_dtypes

import concourse.bass as bass
import concourse.bacc as bacc
import concourse.mybir as mybir
import concourse.tile as tile
from concourse.bass_utils import run_bass_kernel_spmd

B, C, H, W = 8, 64, 64, 64
N = H * W
NP = N // 2          # pixels per parity = 2048
INTER = 16
EPS = 1e-5
JP = W // 2          # pairs per row = 32
RSTR = JP + 2        # buf row stride = 34 (pad_l, 32 slots, pad_r)
NROW = H + 2         # 66 buffered rows (zero row above/below)
BUFCOLS = NROW * RSTR + 4   # 2248 incl. slack
CHUNK = 512
NCHUNK = NP // CHUNK  # 4
ROWS_PER_CHUNK = CHUNK // JP  # 16

BF16 = mybir.dt.bfloat16
F32 = mybir.dt.float32

# blob (bf16) column layout
O_W1B = 0            # [128, 32]
O_W2B = 32           # [32, 128]
O_CONV = 160         # 6 x [128, 128]: A(-1), A(0), A(+1), B(-1), B(0), B(+1)
BLOBCOLS = 160 + 6 * 128     # 928
BLOB_A_COLS = 160    # first DMA: mm1/mm2 weights

# aux (f32) column layout
O_B1 = 0             # rows 0:32
O_GB1 = 1            # rows 0:16
O_BSIG = 2           # rows 0:128
O_CB = 3             # rows 0:128
O_GW1B = 4           # [128, 16]
O_GW2B = 20          # [16, 128]
AUXCOLS = 20 + 128   # 148

N_FILLERS = 4        # PE p-state warmers (scratch matmuls on x data)

_prog_cache = {}


def _row(r):
    """Flat offset of image row r's pad_l in buf1/buf2 (rows -1..64)."""
    return (r + 1) * RSTR


def build_program(n_cores=8):
    nc = bacc.Bacc("TRN2", debug=False, target_bir_lowering=False,
                   num_devices=n_cores)

    xin_d = nc.dram_tensor("xin", [2 * C, NP], BF16, kind="ExternalInput").ap()
    blob_d = nc.dram_tensor("blob", [2 * C, BLOBCOLS], BF16,
                            kind="ExternalInput").ap()
    aux_d = nc.dram_tensor("aux", [2 * C, AUXCOLS], F32,
                           kind="ExternalInput").ap()
    y_d = nc.dram_tensor("y", [2 * C, NP], BF16, kind="ExternalOutput").ap()

    with tile.TileContext(nc) as tc:
        with tc.tile_pool(name="big", bufs=1) as bpool, \
             tc.tile_pool(name="t1p", bufs=4) as t1pool, \
             tc.tile_pool(name="sigp", bufs=2) as sigpool, \
             tc.tile_pool(name="ybp", bufs=2) as ybpool, \
             tc.tile_pool(name="ps1p", bufs=1, space="PSUM") as pp1, \
             tc.tile_pool(name="ps2p", bufs=2, space="PSUM") as pp2, \
             tc.tile_pool(name="psyp", bufs=4, space="PSUM") as ppy:

            xsb = bpool.tile([2 * C, NP], BF16, tag="xsb")
            wsb = bpool.tile([2 * C, BLOBCOLS], BF16, tag="wsb")
            auxsb = bpool.tile([2 * C, AUXCOLS], F32, tag="auxsb")
            buf1 = bpool.tile([2 * C, BUFCOLS], BF16, tag="buf1")
            buf2 = bpool.tile([2 * C, BUFCOLS], BF16, tag="buf2")
            gparts = bpool.tile([2 * C, 2], F32, tag="gparts")
            gsum = bpool.tile([2 * C, 1], F32, tag="gsum")
            g1 = bpool.tile([INTER, 1], F32, tag="g1")
            dbias = bpool.tile([2 * C, 1], F32, tag="dbias")
            scr_in = bpool.tile([1, 2], BF16, tag="scrin")
            scr_out = bpool.tile([1, 2], BF16, tag="scrout")

            # ---- activation-table preload: a dummy sigmoid with no deps
            # makes the (single) table load happen at t~0; the chosen set
            # also contains Relu/Identity so no reload later ----
            nc.vector.memset(scr_in[:].bitcast(mybir.dt.uint16), 0)
            nc.scalar.activation(scr_out[:], scr_in[:],
                                 mybir.ActivationFunctionType.Sigmoid)

            # ---- DMAs: x halves on the SP queue; weights on the
            # Activation-issued queue (parallel DGE pipelines) ----
            nc.sync.dma_start(xsb[:, 0:NP // 2], xin_d[:, 0:NP // 2])
            nc.sync.dma_start(xsb[:, NP // 2:NP], xin_d[:, NP // 2:NP])
            nc.scalar.dma_start(wsb[:, 0:BLOB_A_COLS],
                                blob_d[:, 0:BLOB_A_COLS])
            nc.scalar.dma_start(auxsb[:], aux_d)
            nc.scalar.dma_start(wsb[:, BLOB_A_COLS:BLOBCOLS],
                                blob_d[:, BLOB_A_COLS:BLOBCOLS])

            w1b = wsb[:, O_W1B:O_W1B + 32]
            w2b = wsb[0:32, O_W2B:O_W2B + 128]
            convw = [wsb[:, O_CONV + k * 128:O_CONV + (k + 1) * 128]
                     for k in range(6)]   # A-1 A0 A+1 B-1 B0 B+1
            b1 = auxsb[0:32, O_B1:O_B1 + 1]
            gb1 = auxsb[0:INTER, O_GB1:O_GB1 + 1]
            bsig = auxsb[:, O_BSIG:O_BSIG + 1]
            cb = auxsb[:, O_CB:O_CB + 1]
            gw1b = auxsb[:, O_GW1B:O_GW1B + INTER]
            gw2b = auxsb[0:INTER, O_GW2B:O_GW2B + 128]

            # ---- zero pads (DVE): rows -1/64 in both bufs + buf1 slot pads
            # (buf1's pads propagate zeros into buf2 via the swap copies) ----
            nc.vector.memset(buf1[:, 0:RSTR].bitcast(mybir.dt.uint16), 0)
            nc.vector.memset(
                buf1[:, _row(H):_row(H) + RSTR].bitcast(mybir.dt.uint16), 0)
            inner = buf1[:, RSTR:RSTR + H * RSTR].rearrange(
                "p (r w) -> p r w", w=RSTR)
            nc.vector.memset(inner[:, :, 0:1].bitcast(mybir.dt.uint16), 0)
            nc.vector.memset(
                inner[:, :, RSTR - 1:RSTR].bitcast(mybir.dt.uint16), 0)
            nc.vector.memset(buf2[:, 0:RSTR].bitcast(mybir.dt.uint16), 0)
            nc.vector.memset(
                buf2[:, _row(H):_row(H) + RSTR].bitcast(mybir.dt.uint16), 0)

            # ---- global mean partial reduces (DVE), per x DMA half ----
            for hi in range(2):
                nc.vector.reduce_sum(gparts[:, hi:hi + 1],
                                     xsb[:, hi * (NP // 2):(hi + 1) * (NP // 2)],
                                     axis=mybir.AxisListType.X)
            nc.vector.reduce_sum(gsum[:], gparts[:], axis=mybir.AxisListType.X)

            # ---- mm1 all chunks into one PSUM bank (partition-offset);
            # t1relu spread across Act/Pool/Pool/DVE ----
            ps1big = pp1.tile([3 * 32, CHUNK], F32, tag="ps1")
            t1s = {}
            for ci in range(NCHUNK):
                # chunk 3 reuses chunk 0's partitions (WAR after t1relu_0)
                ps1 = ps1big[32 * (ci % 3):32 * (ci % 3) + 32, :]
                nc.tensor.matmul(ps1, w1b,
                                 xsb[:, ci * CHUNK:(ci + 1) * CHUNK],
                                 start=True, stop=True)
                t1 = t1pool.tile([32, CHUNK], BF16, tag="t1")
                t1s[ci] = t1
                if ci == 0:
                    nc.scalar.activation(t1[:], ps1,
                                         mybir.ActivationFunctionType.Relu,
                                         bias=b1)
                elif ci in (1, 2):
                    nc.gpsimd.tensor_scalar(t1[:], ps1, b1, 0.0,
                                            mybir.AluOpType.add,
                                            mybir.AluOpType.max)
                else:
                    nc.vector.tensor_scalar(t1[:], ps1, b1, 0.0,
                                            mybir.AluOpType.add,
                                            mybir.AluOpType.max)

            # ---- global branch: dbias = G2 @ relu(G1 @ mean + gb1) + bsig ----
            ps_g1 = ppy.tile([INTER, 1], F32, tag="psy")
            nc.tensor.matmul(ps_g1[:], gw1b, gsum[:], start=True, stop=True)
            nc.scalar.activation(g1[:], ps_g1[:],
                                 mybir.ActivationFunctionType.Relu,
                                 bias=gb1, scale=1.0 / N)
            ps_g2 = ppy.tile([2 * C, 1], F32, tag="psy")
            nc.tensor.matmul(ps_g2[:], gw2b, g1[:], start=True, stop=True)
            nc.scalar.activation(dbias[:], ps_g2[:],
                                 mybir.ActivationFunctionType.Identity,
                                 bias=bsig)

            # ---- mm2 + sigmoid + gated mul into buf1, swap into buf2 ----
            def fill():
                psf = ppy.tile([2 * C, CHUNK], F32, tag="psy")
                nc.tensor.matmul(psf[:], xsb[:, 0:128], xsb[:, 0:CHUNK],
                                 start=True, stop=True)

            for ci in range(NCHUNK):
                ps2 = pp2.tile([2 * C, CHUNK], F32, tag="ps2")
                nc.tensor.matmul(ps2[:], w2b, t1s[ci][:],
                                 start=True, stop=True)
                if N_FILLERS > ci:
                    fill()
                sig = sigpool.tile([2 * C, CHUNK], BF16, tag="sig")
                nc.scalar.activation(sig[:], ps2[:],
                                     mybir.ActivationFunctionType.Sigmoid,
                                     bias=dbias[:])
                r0 = ci * ROWS_PER_CHUNK
                dst = buf1[:, _row(r0):_row(r0) + ROWS_PER_CHUNK * RSTR]
                dst = dst.rearrange("p (r w) -> p r w", w=RSTR)[:, :, 1:JP + 1]
                xcr = xsb[:, ci * CHUNK:(ci + 1) * CHUNK].rearrange(
                    "p (r w) -> p r w", w=JP)
                sgr = sig[:].rearrange("p (r w) -> p r w", w=JP)
                nc.vector.tensor_mul(dst, xcr, sgr)
                # swap halves into buf2 (flat contiguous, pads carry zeros):
                #   buf2_low slot j = xo_odd[j-1]; buf2_high slot j = xo_even[j+1]
                # low half issued from the Act HWDGE queue, high from SP --
                # two parallel DGE pipelines.
                s0 = _row(r0) + 1
                ln = ROWS_PER_CHUNK * RSTR
                # chunk 0 starts one element earlier so buf2_low row0/slot0
                # picks up buf1's zero pad
                ext = 1 if ci == 0 else 0
                nc.scalar.dma_start(buf2[0:C, s0 + 1 - ext:s0 + 1 + ln],
                                    buf1[C:2 * C, s0 - ext:s0 + ln])
                nc.sync.dma_start(buf2[C:2 * C, s0 - 1:s0 - 1 + ln],
                                  buf1[0:C, s0:s0 + ln])

            # ---- conv: 3 dense A matmuls + 3 half B matmuls per chunk ----
            def rhs(buf, r0, dy):
                v = buf[:, _row(r0 + dy) + 1:
                        _row(r0 + dy) + 1 + ROWS_PER_CHUNK * RSTR]
                return v.rearrange("p (r w) -> p r w", w=RSTR)[:, :, 0:JP]

            psys = {}
            for ci in range(NCHUNK):
                r0 = ci * ROWS_PER_CHUNK
                psy = ppy.tile([2 * C, CHUNK], F32, tag="psy")
                psys[ci] = psy
                for j, dy in enumerate((-1, 0, 1)):
                    nc.tensor.matmul(psy[:], convw[j], rhs(buf1, r0, dy),
                                     start=(j == 0), stop=False)
            for ci in range(NCHUNK):
                r0 = ci * ROWS_PER_CHUNK
                psy = psys[ci]
                for j, dy in enumerate((-1, 0, 1)):
                    nc.tensor.matmul(psy[:], convw[3 + j], rhs(buf2, r0, dy),
                                     start=False, stop=(j == 2))
                ybuf = ybpool.tile([2 * C, CHUNK], BF16, tag="ybuf")
                if ci % 2 == 0:
                    nc.scalar.activation(ybuf[:], psy[:],
                                         mybir.ActivationFunctionType.Relu,
                                         bias=cb)
                    nc.sync.dma_start(y_d[:, ci * CHUNK:(ci + 1) * CHUNK],
                                      ybuf[:])
                else:
                    nc.vector.tensor_scalar(ybuf[:], psy[:], cb, 0.0,
                                            mybir.AluOpType.add,
                                            mybir.AluOpType.max)
                    nc.scalar.dma_start(y_d[:, ci * CHUNK:(ci + 1) * CHUNK],
                                        ybuf[:])

    nc.compile()
    return nc


def _affine(s, b, m, v):
    inv = s / np.sqrt(v + EPS)
    return inv, b - m * inv


def prepare_weights(inputs):
    f = lambda k: np.asarray(inputs[k], dtype=np.float32)
    a1, c1 = _affine(f("ls1"), f("lbb1"), f("lm1"), f("lv1"))
    W1 = a1[:, None] * f("lw1")                    # [16, 64]
    B1 = a1 * f("lb1") + c1
    a2, c2 = _affine(f("ls2"), f("lbb2"), f("lm2"), f("lv2"))
    W2 = a2[:, None] * f("lw2")                    # [64, 16]
    B2 = a2 * f("lb2") + c2
    ag1, cg1 = _affine(f("gs1"), f("gbb1"), f("gm1"), f("gv1"))
    G1 = ag1[:, None] * f("gw1")                   # [16, 64]
    Bg1 = ag1 * f("gb1") + cg1
    ag2, cg2 = _affine(f("gs2"), f("gbb2"), f("gm2"), f("gv2"))
    G2 = ag2[:, None] * f("gw2")                   # [64, 16]
    Bg2 = ag2 * f("gb2") + cg2
    ac, cc = _affine(f("cs"), f("cbb"), f("cm"), f("cv"))
    CW = ac[:, None, None, None] * f("cw")         # [O, C, 3, 3]
    CB = ac * f("cb") + cc
    return dict(W1=W1, B1=B1, W2=W2, G1=G1, Bg1=Bg1, G2=G2,
                bsig=B2 + Bg2, CW=CW, CB=CB)


def assemble_blob(sh):
    blob = np.zeros((2 * C, BLOBCOLS), np.float32)
    W1T = sh["W1"].T                               # [64, 16]
    blob[0:C, O_W1B:O_W1B + INTER] = W1T
    blob[C:2 * C, O_W1B + INTER:O_W1B + 32] = W1T
    W2T = sh["W2"].T                               # [16, 64]
    blob[0:INTER, O_W2B:O_W2B + C] = W2T
    blob[INTER:32, O_W2B + C:O_W2B + 2 * C] = W2T
    CW = sh["CW"]
    cwt = lambda dy, dx: CW[:, :, dy + 1, dx + 1].T   # [c, o]
    for j, dy in enumerate((-1, 0, 1)):
        A = np.zeros((2 * C, 2 * C), np.float32)
        A[0:C, 0:C] = cwt(dy, 0)
        A[C:2 * C, 0:C] = cwt(dy, 1)
        A[0:C, C:2 * C] = cwt(dy, -1)
        A[C:2 * C, C:2 * C] = cwt(dy, 0)
        blob[:, O_CONV + j * 128:O_CONV + (j + 1) * 128] = A
        Bm = np.zeros((2 * C, 2 * C), np.float32)
        Bm[0:C, 0:C] = cwt(dy, -1)
        Bm[C:2 * C, C:2 * C] = cwt(dy, 1)
        blob[:, O_CONV + (3 + j) * 128:O_CONV + (4 + j) * 128] = Bm
    return blob.astype(ml_dtypes.bfloat16)


def assemble_aux(sh):
    aux = np.zeros((2 * C, AUXCOLS), np.float32)
    aux[0:INTER, O_B1] = sh["B1"]
    aux[INTER:32, O_B1] = sh["B1"]
    aux[0:INTER, O_GB1] = sh["Bg1"]
    aux[0:C, O_BSIG] = sh["bsig"]
    aux[C:2 * C, O_BSIG] = sh["bsig"]
    aux[0:C, O_CB] = sh["CB"]
    aux[C:2 * C, O_CB] = sh["CB"]
    G1T = sh["G1"].T                               # [64, 16]
    aux[0:C, O_GW1B:O_GW1B + INTER] = G1T
    aux[C:2 * C, O_GW1B:O_GW1B + INTER] = G1T
    G2T = sh["G2"].T                               # [16, 64]
    aux[0:INTER, O_GW2B:O_GW2B + C] = G2T
    aux[0:INTER, O_GW2B + C:O_GW2B + 2 * C] = G2T
    return np.ascontiguousarray(aux)


def pack_x(xi):
    """[C, H, W] f32 -> [128, NP] bf16 parity-split."""
    ev = xi[:, :, 0::2].reshape(C, NP)
    od = xi[:, :, 1::2].reshape(C, NP)
    return np.ascontiguousarray(
        np.concatenate([ev, od], axis=0)).astype(ml_dtypes.bfloat16)


def unpack_y(yc):
    """[128, NP] bf16 -> [C, H, W] f32."""
    y = np.empty((C, H, W), np.float32)
    y[:, :, 0::2] = np.asarray(yc[0:C], np.float32).reshape(C, H, JP)
    y[:, :, 1::2] = np.asarray(yc[C:2 * C], np.float32).reshape(C, H, JP)
    return y


def make_core_inputs(inputs):
    sh = prepare_weights(inputs)
    blob = assemble_blob(sh)
    aux = assemble_aux(sh)
    x = np.asarray(inputs["x"], dtype=np.float32)
    return [{"xin": pack_x(x[i]), "blob": blob, "aux": aux}
            for i in range(B)]


def _run(inputs, trace=False):
    in_maps = make_core_inputs(inputs)
    if "prog" not in _prog_cache:
        _prog_cache["prog"] = build_program(B)
    nc = _prog_cache["prog"]
    res = run_bass_kernel_spmd(nc, in_maps, list(range(B)), trace=trace)
    out = np.stack([unpack_y(r["y"]) for r in res.results])
    return out.astype(np.float32), res


def kernel(**inputs):
    out, _ = _run(inputs, trace=False)
    return out


def kernel_traced(inputs):
    return _run(inputs, trace=True)


def reference_numpy(inputs):
    """Numpy emulation of the device algebra (parity layout, bf16 casts)."""
    bf = lambda a: a.astype(ml_dtypes.bfloat16).astype(np.float32)
    sh = prepare_weights(inputs)
    blob = np.asarray(assemble_blob(sh), np.float32)
    aux = assemble_aux(sh)
    x = np.asarray(inputs["x"], dtype=np.float32)
    out = np.empty_like(x)
    w1b = blob[:, O_W1B:O_W1B + 32]
    w2b = blob[0:32, O_W2B:O_W2B + 128]
    convw = [blob[:, O_CONV + k * 128:O_CONV + (k + 1) * 128]
             for k in range(6)]
    for i in range(B):
        xp = np.asarray(pack_x(x[i]), np.float32)      # [128, NP]
        gs = xp.sum(axis=1, keepdims=True)             # [128, 1]
        g1 = np.maximum(aux[:, O_GW1B:O_GW1B + INTER][0:128].T @ gs / N
                        + aux[0:INTER, O_GB1:O_GB1 + 1], 0.0)
        db = aux[0:INTER, O_GW2B:O_GW2B + 128].T @ g1 \
            + aux[:, O_BSIG:O_BSIG + 1]
        t1 = bf(np.maximum(w1b.T @ xp + aux[0:32, O_B1:O_B1 + 1], 0.0))
        sarg = w2b.T @ t1 + db
        sig = bf(1.0 / (1.0 + np.exp(-sarg)))
        xo = bf(xp * sig)
        # padded buffers
        b1_ = np.zeros((128, NROW * RSTR + 4), np.float32)
        v = b1_[:, RSTR:RSTR + H * RSTR].reshape(128, H, RSTR)
        v[:, :, 1:JP + 1] = xo.reshape(128, H, JP)
        b2_ = np.zeros_like(b1_)
        s0 = RSTR + 1
        ln = H * RSTR
        b2_[0:C, s0 + 1:s0 + 1 + ln] = b1_[C:2 * C, s0:s0 + ln]
        b2_[C:2 * C, s0 - 1:s0 - 1 + ln] = b1_[0:C, s0:s0 + ln]
        y = np.zeros((128, NP), np.float32)
        for j, dy in enumerate((-1, 0, 1)):
            for bb, wb in ((b1_, convw[j]), (b2_, convw[3 + j])):
                sh_v = bb[:, (1 + dy) * RSTR + 1:
                          (1 + dy) * RSTR + 1 + H * RSTR]
                sh_v = sh_v.reshape(128, H, RSTR)[:, :, 0:JP].reshape(128, NP)
                y += wb.T @ bf(sh_v)
        y = np.maximum(y + aux[:, O_CB:O_CB + 1], 0.0)
        out[i] = unpack_y(y.astype(ml_dtypes.bfloat16))
    return out
